# revision 17
# baseline (speedup 1.0000x reference)
"""Trainium2 Bass kernel for nn_BernoulliEdge (gnn_message_passing).

Math (see reference):
  probs[b,i] = clip(sigmoid(leaky_relu(concat(nodes[b,nn_b], nodes[b,i]) @ W1 + b1) @ W2 + b2))
  w = zeros except w[b, i<nn_b, nn_b] = probs[b,i]           (weights input is all-zero)
  adj_out = hard gumbel-softmax bit of stacked (1-w, w) logits + fixed seed-42 noise.

Since the gumbel noise is a fixed constant and weights==0, adj_out is a constant
{0,1} pattern except in column nn_b (rows < nn_b) of each batch.  The constant
pattern is computed once on the host with the exact reference expression (same
jax default backend => bit-identical ties); the per-call work (edge MLP, column
merge, and materializing the 2x64MB outputs) runs on 8 NeuronCores via Bass,
batch-parallel (2 batches per core).
"""

import numpy as np

B, N, F = 16, 1024, 128
NCORES = 8
BPC = B // NCORES  # batches per core
CLAMP_LO, CLAMP_HI = 0.001, 0.999
GUMBEL_SEED = 42

# smalls packing (one [128, SC] f32 tensor per core)
SC_B1 = 0
SC_W2 = 1
SC_B2 = 2
SC_LEFT = 4   # +b
SC_JCOL = 6   # +b
SC_BLK = 8    # per-batch [128,8] blocks: g0, g1, basecol, imask
SC_PER_B = 32
SC = SC_BLK + BPC * SC_PER_B  # 72

_STATE = {}


def _consts():
    """Constant part of adj_out (w=0 everywhere) + gumbel planes.

    Verbatim mirror of the reference's sample_hard with w=0, evaluated with
    jnp on the default backend so ties break identically to the oracle.
    """
    import jax
    import jax.numpy as jnp

    w = jnp.zeros((B, N, N), jnp.float32)
    logits = jnp.log(jnp.clip(jnp.stack([1.0 - w, w], axis=0), CLAMP_LO, CLAMP_HI))
    u = jax.random.uniform(jax.random.key(GUMBEL_SEED), logits.shape,
                           minval=1e-10, maxval=1.0)
    g = -jnp.log(-jnp.log(u))
    y_soft = jax.nn.softmax(logits + g, axis=0)
    hard1 = jnp.argmax(y_soft, axis=0) == 1
    base = np.asarray(hard1).astype(np.uint8)
    g = np.asarray(g)
    return base, np.ascontiguousarray(g[0]), np.ascontiguousarray(g[1])


def _build_nc(reps=1, chunk=2):
    import concourse.bacc as bacc
    import concourse.mybir as mybir
    from concourse.tile import TileContext

    f32 = mybir.dt.float32
    u8 = mybir.dt.uint8
    Alu = mybir.AluOpType
    Act = mybir.ActivationFunctionType

    import concourse.bass as bass

    nc = bacc.Bacc()
    base_d = nc.dram_tensor("base", [BPC, N, N], u8, kind="ExternalInput")
    jidx_d = nc.dram_tensor("jidx", [1, BPC], mybir.dt.int32, kind="ExternalInput")
    nodesT_d = nc.dram_tensor("nodesT", [BPC, F, N], f32, kind="ExternalInput")
    w1_d = nc.dram_tensor("w1", [2 * F, F], f32, kind="ExternalInput")
    smalls_d = nc.dram_tensor("smalls", [128, SC], f32, kind="ExternalInput")
    tick_d = nc.dram_tensor("tick", [1, 1], f32, kind="ExternalInput")
    adj_d = nc.dram_tensor("adj", [BPC, N, N], f32, kind="ExternalOutput")
    w_d = nc.dram_tensor("w", [BPC, N, N], f32, kind="ExternalOutput")
    tock_d = nc.dram_tensor("tock", [1, 1], f32, kind="ExternalOutput")

    with TileContext(nc) as tc:
        with tc.tile_pool(name="sb", bufs=1) as cpool, \
             tc.tile_pool(name="big", bufs=2) as bigpool, \
             tc.tile_pool(name="adjt", bufs=4) as adjpool, \
             tc.tile_pool(name="wt", bufs=4) as wpool, \
             tc.tile_pool(name="col", bufs=1) as colpool, \
             tc.tile_pool(name="ps_h", bufs=2, space="PSUM") as ps_h, \
             tc.tile_pool(name="ps_s", bufs=2, space="PSUM") as ps_s:

            # tick passthrough (lets callers chain executions for timing)
            tick_t = cpool.tile([1, 1], f32)
            nc.sync.dma_start(out=tick_t, in_=tick_d[:, :])
            nc.vector.tensor_scalar_add(tick_t, tick_t, 1.0)
            nc.sync.dma_start(out=tock_d[:, :], in_=tick_t)

            smalls = cpool.tile([128, SC], f32)
            nc.sync.dma_start(out=smalls, in_=smalls_d[:, :])
            w1a = cpool.tile([128, F], f32)
            nc.sync.dma_start(out=w1a, in_=w1_d[0:F, :])
            w1b = cpool.tile([128, F], f32)
            nc.sync.dma_start(out=w1b, in_=w1_d[F:2 * F, :])
            iota_t = cpool.tile([128, N], f32)
            nc.gpsimd.iota(iota_t, pattern=[[1, N]], base=0, channel_multiplier=0,
                           allow_small_or_imprecise_dtypes=True)
            ones8 = cpool.tile([128, 8], f32)
            nc.vector.memset(ones8, 1.0)
            jidx_t = cpool.tile([1, BPC], mybir.dt.int32)
            nc.sync.dma_start(out=jidx_t, in_=jidx_d[:, :])
            jreg = nc.sync.register(name="jreg").__enter__()

            for b in [b for _ in range(reps) for b in range(BPC)]:
                blk = SC_BLK + b * SC_PER_B
                g0col = smalls[:, blk:blk + 8]
                g1col = smalls[:, blk + 8:blk + 16]
                basecol = smalls[:, blk + 16:blk + 24]
                imask = smalls[:, blk + 24:blk + 32]

                nodesT = bigpool.tile([128, N], f32, tag="nodesT")
                nc.sync.dma_start(out=nodesT, in_=nodesT_d[b, :, :])

                # bias_col = W1a.T @ leftT + b1
                psb = ps_s.tile([128, 1], f32, tag="psb")
                nc.tensor.matmul(psb, w1a, smalls[:, SC_LEFT + b:SC_LEFT + b + 1],
                                 start=True, stop=True)
                biascol = colpool.tile([128, 1], f32, tag="biascol")
                nc.vector.tensor_tensor(biascol, psb, smalls[:, SC_B1:SC_B1 + 1],
                                        Alu.add)

                # H.T = leaky_relu(W1b.T @ nodesT + bias_col)
                h_sb = bigpool.tile([128, N], f32, tag="h_sb")
                for hh in range(2):
                    ph = ps_h.tile([128, 512], f32, tag="ph")
                    nc.tensor.matmul(ph, w1b, nodesT[:, hh * 512:(hh + 1) * 512],
                                     start=True, stop=True)
                    nc.scalar.activation(h_sb[:, hh * 512:(hh + 1) * 512], ph,
                                         Act.Lrelu, bias=biascol[:, 0:1],
                                         scale=1.0, alpha=0.01)

                # logits[i] = H @ W2, packed [128 x 8] with i = t*128 + p
                pp = ps_s.tile([128, 8], f32, tag="pp")
                for t in range(8):
                    nc.tensor.matmul(pp[:, t:t + 1],
                                     h_sb[:, t * 128:(t + 1) * 128],
                                     smalls[:, SC_W2:SC_W2 + 1],
                                     start=True, stop=True)

                probs = colpool.tile([128, 8], f32, tag="probs")
                nc.scalar.activation(probs, pp, Act.Sigmoid,
                                     bias=smalls[:, SC_B2:SC_B2 + 1], scale=1.0)
                pc = colpool.tile([128, 8], f32, tag="pc")
                nc.vector.tensor_scalar_min(pc, probs, CLAMP_HI)
                nc.vector.tensor_scalar_max(pc, pc, CLAMP_LO)
                onem = colpool.tile([128, 8], f32, tag="onem")
                nc.vector.scalar_tensor_tensor(onem, pc, -1.0, ones8,
                                               op0=Alu.mult, op1=Alu.add)
                nc.vector.tensor_scalar_min(onem, onem, CLAMP_HI)
                nc.vector.tensor_scalar_max(onem, onem, CLAMP_LO)
                l1 = colpool.tile([128, 8], f32, tag="l1")
                nc.scalar.activation(l1, pc, Act.Ln, bias=0.0, scale=1.0)
                l0 = colpool.tile([128, 8], f32, tag="l0")
                nc.scalar.activation(l0, onem, Act.Ln, bias=0.0, scale=1.0)
                a1 = colpool.tile([128, 8], f32, tag="a1")
                nc.vector.tensor_tensor(a1, l1, g1col, Alu.add)
                a0 = colpool.tile([128, 8], f32, tag="a0")
                nc.vector.tensor_tensor(a0, l0, g0col, Alu.add)
                hard = colpool.tile([128, 8], f32, tag="hard")
                nc.vector.tensor_tensor(hard, a1, a0, Alu.is_gt)
                # dcol = imask * (hard - basecol); aw = imask * pc
                dcol = colpool.tile([128, 8], f32, tag="dcol")
                nc.vector.tensor_tensor(dcol, hard, basecol, Alu.subtract)
                nc.vector.tensor_tensor(dcol, dcol, imask, Alu.mult)
                aw = colpool.tile([128, 8], f32, tag="aw")
                nc.vector.tensor_tensor(aw, pc, imask, Alu.mult)

                onehot = bigpool.tile([128, N], mybir.dt.bfloat16, tag="onehot")
                nc.vector.tensor_scalar(onehot, iota_t,
                                        smalls[:, SC_JCOL + b:SC_JCOL + b + 1],
                                        None, Alu.is_equal)
                dcol_bf = colpool.tile([128, 8], mybir.dt.bfloat16, tag="dcol_bf")
                nc.vector.tensor_copy(dcol_bf, dcol)

                # base/adj/w planes in chunks of `chunk` 128-row blocks
                base_r = base_d[b, :, :].rearrange("(c p) n -> p c n", p=128)
                adj_r = adj_d[b, :, :].rearrange("(c p) n -> p c n", p=128)
                w_r = w_d[b, :, :].rearrange("(c p) n -> p c n", p=128)
                # w output: only column j* is nonzero; run_bass_kernel_spmd
                # pre-zeroes ExternalOutput buffers, so scatter just the column
                # (1024 strided f32 writes at a runtime offset).
                nc.sync.reg_load(jreg, jidx_t[0:1, b:b + 1])
                jval = nc.sync.snap(jreg)
                w_col = w_d[b, :, :].rearrange(
                    "(t p) n -> p t n", p=128)[:, :, bass.ds(jval, 1)]
                nc.sync.dma_start(out=w_col, in_=aw)

                for t0 in range(0, 8, chunk):
                    bt = adjpool.tile([128, chunk * N], mybir.dt.bfloat16, tag="bt")
                    nc.gpsimd.dma_start(out=bt, in_=base_r[:, t0:t0 + chunk, :])
                    ot = wpool.tile([128, chunk * N], f32, tag="ot")
                    for i in range(chunk):
                        t = t0 + i
                        nc.vector.scalar_tensor_tensor(
                            ot[:, i * N:(i + 1) * N], onehot, dcol_bf[:, t:t + 1],
                            bt[:, i * N:(i + 1) * N], op0=Alu.mult, op1=Alu.add)
                    eng = nc.sync if (t0 // chunk) % 2 == 0 else nc.scalar
                    eng.dma_start(out=adj_r[:, t0:t0 + chunk, :], in_=ot)
    nc.compile()
    return nc


def _ensure_setup():
    if "nc" in _STATE:
        return
    base, g0, g1 = _consts()
    _STATE["base"] = base
    _STATE["g0"] = g0
    _STATE["g1"] = g1
    _STATE["nc"] = _build_nc()


def _mirror(nodes, adj, weights, num_nodes, B_, W1, b1, W2, b2):
    """Full verbatim jnp fallback (used only for unexpected inputs)."""
    import jax
    import jax.numpy as jnp

    nodes = jnp.asarray(nodes)
    weights = jnp.asarray(weights)
    num_nodes = jnp.asarray(num_nodes)
    Bn, Nn, Fn = nodes.shape
    left = nodes[jnp.arange(Bn), num_nodes]
    net_in = jnp.concatenate(
        [jnp.broadcast_to(left[:, None, :], (Bn, Nn, Fn)), nodes], axis=-1)
    h = jax.nn.leaky_relu(net_in @ jnp.asarray(W1) + jnp.asarray(b1))
    probs = jnp.clip(jax.nn.sigmoid(h @ jnp.asarray(W2) + jnp.asarray(b2))[..., 0],
                     CLAMP_LO, CLAMP_HI)
    i_mask = jnp.arange(Nn)[None, :] < num_nodes[:, None]
    j_onehot = jnp.arange(Nn)[None, None, :] == num_nodes[:, None, None]
    sel = i_mask[:, :, None] & j_onehot
    w = jnp.where(sel, probs[:, :, None], weights)
    logits = jnp.log(jnp.clip(jnp.stack([1.0 - w, w], axis=0), CLAMP_LO, CLAMP_HI))
    u = jax.random.uniform(jax.random.key(GUMBEL_SEED), logits.shape,
                           minval=1e-10, maxval=1.0)
    g = -jnp.log(-jnp.log(u))
    y_soft = jax.nn.softmax(logits + g, axis=0)
    hard1 = (jnp.argmax(y_soft, axis=0) == 1).astype(w.dtype)
    adj_out = hard1 + y_soft[1] - jax.lax.stop_gradient(y_soft[1])
    return np.asarray(adj_out), np.asarray(w)


def _make_in_maps(nodes, num_nodes_np, W1np, b1np, W2np, b2np):
    base, g0, g1 = _STATE["base"], _STATE["g0"], _STATE["g1"]
    arange = np.arange(N)
    in_maps = []
    for c in range(NCORES):
        gb0 = c * BPC
        smalls = np.zeros((128, SC), np.float32)
        smalls[:, SC_B1] = b1np.reshape(F)
        smalls[:, SC_W2] = W2np.reshape(F)
        smalls[:, SC_B2] = float(b2np.reshape(-1)[0]) if b2np.size else 0.0
        for b in range(BPC):
            gb = gb0 + b
            j = int(num_nodes_np[gb])
            smalls[:, SC_LEFT + b] = nodes[gb, j, :]
            smalls[:, SC_JCOL + b] = float(j)
            blk = SC_BLK + b * SC_PER_B
            smalls[:, blk:blk + 8] = g0[gb, :, j].reshape(8, 128).T
            smalls[:, blk + 8:blk + 16] = g1[gb, :, j].reshape(8, 128).T
            smalls[:, blk + 16:blk + 24] = \
                base[gb, :, j].astype(np.float32).reshape(8, 128).T
            smalls[:, blk + 24:blk + 32] = \
                (arange < j).astype(np.float32).reshape(8, 128).T
        in_maps.append({
            "base": base[gb0:gb0 + BPC],
            "nodesT": np.ascontiguousarray(
                nodes[gb0:gb0 + BPC].transpose(0, 2, 1)),
            "w1": W1np,
            "smalls": smalls,
            "jidx": num_nodes_np[gb0:gb0 + BPC].astype(np.int32).reshape(1, BPC),
            "tick": np.zeros((1, 1), np.float32),
        })
    return in_maps


def kernel(nodes, adj, weights, num_nodes, B=None, W1=None, b1=None, W2=None,
           b2=None):
    from concourse.bass_utils import run_bass_kernel_spmd

    nodes = np.asarray(nodes, dtype=np.float32)
    num_nodes_np = np.asarray(num_nodes).astype(np.int64)
    W1np = np.asarray(W1, dtype=np.float32)
    b1np = np.asarray(b1, dtype=np.float32)
    W2np = np.asarray(W2, dtype=np.float32)
    b2np = np.asarray(b2, dtype=np.float32)

    if (nodes.shape != (16, N, F) or np.asarray(weights).any()
            or num_nodes_np.min() < 0 or num_nodes_np.max() >= N):
        return _mirror(nodes, adj, weights, num_nodes_np, B, W1np, b1np, W2np,
                       b2np)

    _ensure_setup()
    in_maps = _make_in_maps(nodes, num_nodes_np, W1np, b1np, W2np, b2np)
    res = run_bass_kernel_spmd(_STATE["nc"], in_maps,
                               core_ids=list(range(NCORES)))
    adj_out = np.concatenate([r["adj"] for r in res.results], axis=0)
    w_out = np.concatenate([r["w"] for r in res.results], axis=0)
    return adj_out, w_out


# revision 19
# speedup vs baseline: 3.1720x; 3.1720x over previous
"""Trainium2 Bass kernel for nn_BernoulliEdge (gnn_message_passing).

Math (see reference):
  probs[b,i] = clip(sigmoid(leaky_relu(concat(nodes[b,nn_b], nodes[b,i]) @ W1 + b1) @ W2 + b2))
  w = zeros except w[b, i<nn_b, nn_b] = probs[b,i]           (weights input is all-zero)
  adj_out = hard gumbel-softmax bit of stacked (1-w, w) logits + fixed seed-42 noise.

Since the gumbel noise is a fixed constant and weights==0, adj_out is a constant
{0,1} pattern except in column nn_b (rows < nn_b) of each batch.  The constant
pattern is computed once on the host with the exact reference expression (same
jax default backend => bit-identical ties); the per-call work (edge MLP, column
merge, and materializing the 2x64MB outputs) runs on 8 NeuronCores via Bass,
batch-parallel (2 batches per core).
"""

import numpy as np

B, N, F = 16, 1024, 128
NCORES = 8
BPC = B // NCORES  # batches per core
CLAMP_LO, CLAMP_HI = 0.001, 0.999
GUMBEL_SEED = 42

# smalls packing (one [128, SC] f32 tensor per core)
SC_B1 = 0
SC_W2 = 1
SC_B2 = 2
SC_LEFT = 4   # +b
SC_JCOL = 6   # +b
SC_BLK = 8    # per-batch [128,8] blocks: g0, g1, basecol, imask
SC_PER_B = 32
SC = SC_BLK + BPC * SC_PER_B  # 72

_STATE = {}


def _consts():
    """Constant part of adj_out (w=0 everywhere) + gumbel planes.

    Verbatim mirror of the reference's sample_hard with w=0, evaluated with
    jnp on the default backend so ties break identically to the oracle.
    """
    import jax
    import jax.numpy as jnp

    w = jnp.zeros((B, N, N), jnp.float32)
    logits = jnp.log(jnp.clip(jnp.stack([1.0 - w, w], axis=0), CLAMP_LO, CLAMP_HI))
    u = jax.random.uniform(jax.random.key(GUMBEL_SEED), logits.shape,
                           minval=1e-10, maxval=1.0)
    g = -jnp.log(-jnp.log(u))
    y_soft = jax.nn.softmax(logits + g, axis=0)
    hard1 = jnp.argmax(y_soft, axis=0) == 1
    base = np.asarray(hard1).astype(np.uint8)
    g = np.asarray(g)
    return base, np.ascontiguousarray(g[0]), np.ascontiguousarray(g[1])


def _build_nc(reps=1, chunk=2):
    import concourse.bacc as bacc
    import concourse.mybir as mybir
    from concourse.tile import TileContext

    f32 = mybir.dt.float32
    u8 = mybir.dt.uint8
    Alu = mybir.AluOpType
    Act = mybir.ActivationFunctionType

    import concourse.bass as bass

    nc = bacc.Bacc()
    base_d = nc.dram_tensor("base", [BPC, N, N], u8, kind="ExternalInput")
    jidx_d = nc.dram_tensor("jidx", [1, BPC], mybir.dt.int32, kind="ExternalInput")
    nodesT_d = nc.dram_tensor("nodesT", [BPC, F, N], f32, kind="ExternalInput")
    w1_d = nc.dram_tensor("w1", [2 * F, F], f32, kind="ExternalInput")
    smalls_d = nc.dram_tensor("smalls", [128, SC], f32, kind="ExternalInput")
    tick_d = nc.dram_tensor("tick", [1, 1], f32, kind="ExternalInput")
    adj_d = nc.dram_tensor("adj", [BPC, N, N], f32, kind="ExternalOutput")
    w_d = nc.dram_tensor("w", [BPC, N, N], f32, kind="ExternalOutput")
    tock_d = nc.dram_tensor("tock", [1, 1], f32, kind="ExternalOutput")

    with TileContext(nc) as tc:
        with tc.tile_pool(name="sb", bufs=1) as cpool, \
             tc.tile_pool(name="big", bufs=2) as bigpool, \
             tc.tile_pool(name="adjt", bufs=4) as adjpool, \
             tc.tile_pool(name="wt", bufs=4) as wpool, \
             tc.tile_pool(name="col", bufs=1) as colpool, \
             tc.tile_pool(name="ps_h", bufs=2, space="PSUM") as ps_h, \
             tc.tile_pool(name="ps_s", bufs=2, space="PSUM") as ps_s:

            # tick passthrough (lets callers chain executions for timing)
            tick_t = cpool.tile([1, 1], f32)
            nc.sync.dma_start(out=tick_t, in_=tick_d[:, :])
            nc.vector.tensor_scalar_add(tick_t, tick_t, 1.0)
            nc.sync.dma_start(out=tock_d[:, :], in_=tick_t)

            smalls = cpool.tile([128, SC], f32)
            nc.sync.dma_start(out=smalls, in_=smalls_d[:, :])
            w1a = cpool.tile([128, F], f32)
            nc.sync.dma_start(out=w1a, in_=w1_d[0:F, :])
            w1b = cpool.tile([128, F], f32)
            nc.sync.dma_start(out=w1b, in_=w1_d[F:2 * F, :])
            iota_t = cpool.tile([128, N], f32)
            nc.gpsimd.iota(iota_t, pattern=[[1, N]], base=0, channel_multiplier=0,
                           allow_small_or_imprecise_dtypes=True)
            ones8 = cpool.tile([128, 8], f32)
            nc.vector.memset(ones8, 1.0)
            jidx_t = cpool.tile([1, BPC], mybir.dt.int32)
            nc.sync.dma_start(out=jidx_t, in_=jidx_d[:, :])
            jreg = nc.sync.register(name="jreg").__enter__()

            for b in [b for _ in range(reps) for b in range(BPC)]:
                blk = SC_BLK + b * SC_PER_B
                g0col = smalls[:, blk:blk + 8]
                g1col = smalls[:, blk + 8:blk + 16]
                basecol = smalls[:, blk + 16:blk + 24]
                imask = smalls[:, blk + 24:blk + 32]

                nodesT = bigpool.tile([128, N], f32, tag="nodesT")
                nc.sync.dma_start(out=nodesT, in_=nodesT_d[b, :, :])

                # bias_col = W1a.T @ leftT + b1
                psb = ps_s.tile([128, 1], f32, tag="psb")
                nc.tensor.matmul(psb, w1a, smalls[:, SC_LEFT + b:SC_LEFT + b + 1],
                                 start=True, stop=True)
                biascol = colpool.tile([128, 1], f32, tag="biascol")
                nc.vector.tensor_tensor(biascol, psb, smalls[:, SC_B1:SC_B1 + 1],
                                        Alu.add)

                # H.T = leaky_relu(W1b.T @ nodesT + bias_col)
                h_sb = bigpool.tile([128, N], f32, tag="h_sb")
                for hh in range(2):
                    ph = ps_h.tile([128, 512], f32, tag="ph")
                    nc.tensor.matmul(ph, w1b, nodesT[:, hh * 512:(hh + 1) * 512],
                                     start=True, stop=True)
                    nc.scalar.activation(h_sb[:, hh * 512:(hh + 1) * 512], ph,
                                         Act.Lrelu, bias=biascol[:, 0:1],
                                         scale=1.0, alpha=0.01)

                # logits[i] = H @ W2, packed [128 x 8] with i = t*128 + p
                pp = ps_s.tile([128, 8], f32, tag="pp")
                for t in range(8):
                    nc.tensor.matmul(pp[:, t:t + 1],
                                     h_sb[:, t * 128:(t + 1) * 128],
                                     smalls[:, SC_W2:SC_W2 + 1],
                                     start=True, stop=True)

                probs = colpool.tile([128, 8], f32, tag="probs")
                nc.scalar.activation(probs, pp, Act.Sigmoid,
                                     bias=smalls[:, SC_B2:SC_B2 + 1], scale=1.0)
                pc = colpool.tile([128, 8], f32, tag="pc")
                nc.vector.tensor_scalar_min(pc, probs, CLAMP_HI)
                nc.vector.tensor_scalar_max(pc, pc, CLAMP_LO)
                onem = colpool.tile([128, 8], f32, tag="onem")
                nc.vector.scalar_tensor_tensor(onem, pc, -1.0, ones8,
                                               op0=Alu.mult, op1=Alu.add)
                nc.vector.tensor_scalar_min(onem, onem, CLAMP_HI)
                nc.vector.tensor_scalar_max(onem, onem, CLAMP_LO)
                l1 = colpool.tile([128, 8], f32, tag="l1")
                nc.scalar.activation(l1, pc, Act.Ln, bias=0.0, scale=1.0)
                l0 = colpool.tile([128, 8], f32, tag="l0")
                nc.scalar.activation(l0, onem, Act.Ln, bias=0.0, scale=1.0)
                a1 = colpool.tile([128, 8], f32, tag="a1")
                nc.vector.tensor_tensor(a1, l1, g1col, Alu.add)
                a0 = colpool.tile([128, 8], f32, tag="a0")
                nc.vector.tensor_tensor(a0, l0, g0col, Alu.add)
                hard = colpool.tile([128, 8], f32, tag="hard")
                nc.vector.tensor_tensor(hard, a1, a0, Alu.is_gt)
                # dcol = imask * (hard - basecol); aw = imask * pc
                dcol = colpool.tile([128, 8], f32, tag="dcol")
                nc.vector.tensor_tensor(dcol, hard, basecol, Alu.subtract)
                nc.vector.tensor_tensor(dcol, dcol, imask, Alu.mult)
                aw = colpool.tile([128, 8], f32, tag="aw")
                nc.vector.tensor_tensor(aw, pc, imask, Alu.mult)

                onehot = bigpool.tile([128, N], mybir.dt.bfloat16, tag="onehot")
                nc.vector.tensor_scalar(onehot, iota_t,
                                        smalls[:, SC_JCOL + b:SC_JCOL + b + 1],
                                        None, Alu.is_equal)
                dcol_bf = colpool.tile([128, 8], mybir.dt.bfloat16, tag="dcol_bf")
                nc.vector.tensor_copy(dcol_bf, dcol)

                # base/adj/w planes in chunks of `chunk` 128-row blocks
                base_r = base_d[b, :, :].rearrange("(c p) n -> p c n", p=128)
                adj_r = adj_d[b, :, :].rearrange("(c p) n -> p c n", p=128)
                w_r = w_d[b, :, :].rearrange("(c p) n -> p c n", p=128)
                # w output: only column j* is nonzero; run_bass_kernel_spmd
                # pre-zeroes ExternalOutput buffers, so scatter just the column
                # (1024 strided f32 writes at a runtime offset).
                nc.sync.reg_load(jreg, jidx_t[0:1, b:b + 1])
                jval = nc.sync.snap(jreg)
                w_col = w_d[b, :, :].rearrange(
                    "(t p) n -> p t n", p=128)[:, :, bass.ds(jval, 1)]
                nc.sync.dma_start(out=w_col, in_=aw)

                for t0 in range(0, 8, chunk):
                    bt = adjpool.tile([128, chunk * N], mybir.dt.bfloat16, tag="bt")
                    nc.gpsimd.dma_start(out=bt, in_=base_r[:, t0:t0 + chunk, :])
                    ot = wpool.tile([128, chunk * N], f32, tag="ot")
                    for i in range(chunk):
                        t = t0 + i
                        nc.vector.scalar_tensor_tensor(
                            ot[:, i * N:(i + 1) * N], onehot, dcol_bf[:, t:t + 1],
                            bt[:, i * N:(i + 1) * N], op0=Alu.mult, op1=Alu.add)
                    eng = nc.sync if (t0 // chunk) % 2 == 0 else nc.scalar
                    eng.dma_start(out=adj_r[:, t0:t0 + chunk, :], in_=ot)
    nc.compile()
    return nc


def _ensure_setup():
    if "nc" in _STATE:
        return
    base, g0, g1 = _consts()
    _STATE["base"] = base
    _STATE["g0"] = g0
    _STATE["g1"] = g1
    _STATE["nc"] = _build_nc()


def _mirror(nodes, adj, weights, num_nodes, B_, W1, b1, W2, b2):
    """Full verbatim jnp fallback (used only for unexpected inputs)."""
    import jax
    import jax.numpy as jnp

    nodes = jnp.asarray(nodes)
    weights = jnp.asarray(weights)
    num_nodes = jnp.asarray(num_nodes)
    Bn, Nn, Fn = nodes.shape
    left = nodes[jnp.arange(Bn), num_nodes]
    net_in = jnp.concatenate(
        [jnp.broadcast_to(left[:, None, :], (Bn, Nn, Fn)), nodes], axis=-1)
    h = jax.nn.leaky_relu(net_in @ jnp.asarray(W1) + jnp.asarray(b1))
    probs = jnp.clip(jax.nn.sigmoid(h @ jnp.asarray(W2) + jnp.asarray(b2))[..., 0],
                     CLAMP_LO, CLAMP_HI)
    i_mask = jnp.arange(Nn)[None, :] < num_nodes[:, None]
    j_onehot = jnp.arange(Nn)[None, None, :] == num_nodes[:, None, None]
    sel = i_mask[:, :, None] & j_onehot
    w = jnp.where(sel, probs[:, :, None], weights)
    logits = jnp.log(jnp.clip(jnp.stack([1.0 - w, w], axis=0), CLAMP_LO, CLAMP_HI))
    u = jax.random.uniform(jax.random.key(GUMBEL_SEED), logits.shape,
                           minval=1e-10, maxval=1.0)
    g = -jnp.log(-jnp.log(u))
    y_soft = jax.nn.softmax(logits + g, axis=0)
    hard1 = (jnp.argmax(y_soft, axis=0) == 1).astype(w.dtype)
    adj_out = hard1 + y_soft[1] - jax.lax.stop_gradient(y_soft[1])
    return np.asarray(adj_out), np.asarray(w)


def _make_in_maps(nodes, num_nodes_np, W1np, b1np, W2np, b2np):
    base, g0, g1 = _STATE["base"], _STATE["g0"], _STATE["g1"]
    arange = np.arange(N)
    in_maps = []
    for c in range(NCORES):
        gb0 = c * BPC
        smalls = np.zeros((128, SC), np.float32)
        smalls[:, SC_B1] = b1np.reshape(F)
        smalls[:, SC_W2] = W2np.reshape(F)
        smalls[:, SC_B2] = float(b2np.reshape(-1)[0]) if b2np.size else 0.0
        for b in range(BPC):
            gb = gb0 + b
            j = int(num_nodes_np[gb])
            smalls[:, SC_LEFT + b] = nodes[gb, j, :]
            smalls[:, SC_JCOL + b] = float(j)
            blk = SC_BLK + b * SC_PER_B
            smalls[:, blk:blk + 8] = g0[gb, :, j].reshape(8, 128).T
            smalls[:, blk + 8:blk + 16] = g1[gb, :, j].reshape(8, 128).T
            smalls[:, blk + 16:blk + 24] = \
                base[gb, :, j].astype(np.float32).reshape(8, 128).T
            smalls[:, blk + 24:blk + 32] = \
                (arange < j).astype(np.float32).reshape(8, 128).T
        in_maps.append({
            "base": base[gb0:gb0 + BPC],
            "nodesT": np.ascontiguousarray(
                nodes[gb0:gb0 + BPC].transpose(0, 2, 1)),
            "w1": W1np,
            "smalls": smalls,
            "jidx": num_nodes_np[gb0:gb0 + BPC].astype(np.int32).reshape(1, BPC),
            "tick": np.zeros((1, 1), np.float32),
        })
    return in_maps


def kernel(nodes, adj, weights, num_nodes, B=None, W1=None, b1=None, W2=None,
           b2=None):
    from concourse.bass_utils import run_bass_kernel_spmd

    nodes = np.asarray(nodes, dtype=np.float32)
    num_nodes_np = np.asarray(num_nodes).astype(np.int64)
    W1np = np.asarray(W1, dtype=np.float32)
    b1np = np.asarray(b1, dtype=np.float32)
    W2np = np.asarray(W2, dtype=np.float32)
    b2np = np.asarray(b2, dtype=np.float32)

    if (nodes.shape != (16, N, F) or np.asarray(weights).any()
            or num_nodes_np.min() < 0 or num_nodes_np.max() >= N):
        return _mirror(nodes, adj, weights, num_nodes_np, B, W1np, b1np, W2np,
                       b2np)

    _ensure_setup()
    in_maps = _make_in_maps(nodes, num_nodes_np, W1np, b1np, W2np, b2np)
    res = run_bass_kernel_spmd(_STATE["nc"], in_maps,
                               core_ids=list(range(NCORES)))
    adj_out = np.concatenate([r["adj"] for r in res.results], axis=0)
    w_out = np.concatenate([r["w"] for r in res.results], axis=0)
    return adj_out, w_out
